# revision 36
# baseline (speedup 1.0000x reference)
"""Trainium2 Bass kernel for nn_LocalModel (6-encoder local-attention transformer).

Sharding: data-parallel over batch - B=8 batch elements, one per NeuronCore.
Each core runs the full 6-layer encoder stack + final projection for its
batch element entirely on-chip (all weights resident in SBUF in bf16),
returning a [6]-vector; the host gathers them into the [8, 6] output.

Attention uses the zero-masked-softmax identity: with out-of-window scores
set to 0 (not -inf), softmax over the full sequence satisfies
    out_i = (sum_{j in W} (e^{s_ij} - 1) v_j + sum_all v_j)
          / (sum_{j in W} (e^{s_ij} - 1) + S)
The banded scores are computed qb-centric: key blocks B_j = [128j-64,
128j+64) (tokens padded by 64 zeros each side) against query cols
[128j-128, 128j+128), giving uniform triangular masks. The "-1" term is
folded into the PSUM accumulation via negative-mask matmuls against a
64-shifted V copy (va_shift), so the DVE only does exp-mask multiply.
"""
import sys
import numpy as np

sys.path.insert(0, "/opt/trn_rl_repo")

B, S, D = 8, 1024, 512
H, Dh, W = 8, 64, 64
HD = 2048           # ffn hidden
C = 6               # classes
ENC = 6
EPS = 1e-5
P = 128
KO = D // P         # 4
HC = HD // P        # 16
SCALE = Dh ** -0.5
XW = 64 + S + 64    # padded token width for x / k tiles (1152)
QW = 128 + S + 128  # padded token width for q tiles (1280)

_CACHE = {}
LAST_EXEC_NS = None
LAST_RESULTS = None
TRACE = False


def _build(affine: bool):
    import os
    STAGE = int(os.environ.get("KSTAGE", "9"))
    import concourse.bass as bass
    import concourse.tile as tile
    from concourse import bacc, mybir
    from concourse.masks import make_identity

    f32 = mybir.dt.float32
    bf16 = mybir.dt.bfloat16
    f16 = mybir.dt.float16
    AF = mybir.ActivationFunctionType
    OP = mybir.AluOpType

    nc = bacc.Bacc()
    d = {}
    d['xT'] = nc.declare_dram_parameter("xT", [P, KO, XW], bf16, isOutput=False)
    for w in ("wqT", "wkT", "wvT"):
        d[w] = nc.declare_dram_parameter(w, [P, KO, D], bf16, isOutput=False)
    d['fc1T'] = nc.declare_dram_parameter("fc1T", [P, KO, HD], bf16, isOutput=False)
    d['fc2T'] = nc.declare_dram_parameter("fc2T", [P, HC, D], bf16, isOutput=False)
    d['owT'] = nc.declare_dram_parameter("owT", [P, C, 8, D], bf16, isOutput=False)
    d['bq'] = nc.declare_dram_parameter("bq", [P, KO], f32, isOutput=False)
    d['bk'] = nc.declare_dram_parameter("bk", [P, KO], f32, isOutput=False)
    d['bv'] = nc.declare_dram_parameter("bv", [D], f32, isOutput=False)
    d['bv1k'] = nc.declare_dram_parameter("bv1k", [1, D], f32, isOutput=False)
    d['fc1b'] = nc.declare_dram_parameter("fc1b", [P, HC], f32, isOutput=False)
    d['fc2b'] = nc.declare_dram_parameter("fc2b", [1, D], bf16, isOutput=False)
    # masks: [m_int 256 | m_e0 256 | m_e8 256 | ntri_lo 128 | ntri_lo_e 128
    #         | ntri_up 128 | ntri_up_e 128]  (bf16)
    d['mask'] = nc.declare_dram_parameter("mask", [P, 1280], bf16, isOutput=False)
    if affine:
        d['lng'] = nc.declare_dram_parameter("lng", [D], f32, isOutput=False)
        d['lnb'] = nc.declare_dram_parameter("lnb", [D], f32, isOutput=False)
    out_d = nc.declare_dram_parameter("out", [1, C], f32, isOutput=True)

    def bcast_ap(dram_h, parts=P):
        # replicate a [N] dram vector across `parts` partitions
        a = dram_h[:]
        return bass.AP(tensor=a.tensor, offset=a.offset,
                       ap=[[0, parts]] + [list(x) for x in a.ap])

    def rep_mid(ap2d, reps):
        # [P, N] -> [P, reps, N] with stride-0 middle axis
        return bass.AP(tensor=ap2d.tensor, offset=ap2d.offset,
                       ap=[list(ap2d.ap[0]), [0, reps], list(ap2d.ap[1])])

    def rep_last(ap2d, reps):
        # [P, N] -> [P, N, reps] with stride-0 last axis
        return bass.AP(tensor=ap2d.tensor, offset=ap2d.offset,
                       ap=[list(ap2d.ap[0]), list(ap2d.ap[1]), [0, reps]])

    from contextlib import ExitStack
    with tile.TileContext(nc) as tc, ExitStack() as ctx:
        wpool = ctx.enter_context(tc.tile_pool(name="wpool", bufs=1))
        bigx = ctx.enter_context(tc.tile_pool(name="bigx", bufs=1))
        qkp = ctx.enter_context(tc.tile_pool(name="qkp", bufs=1))
        vap = ctx.enter_context(tc.tile_pool(name="vap", bufs=1))
        pcp = ctx.enter_context(tc.tile_pool(name="pcp", bufs=4))
        atp = ctx.enter_context(tc.tile_pool(name="atp", bufs=2))
        xnp = ctx.enter_context(tc.tile_pool(name="xnp", bufs=1))
        htp = ctx.enter_context(tc.tile_pool(name="htp", bufs=1))
        xxp = ctx.enter_context(tc.tile_pool(name="xxp", bufs=2))
        tmp = ctx.enter_context(tc.tile_pool(name="tmp", bufs=3))
        small = ctx.enter_context(tc.tile_pool(name="small", bufs=4))
        psA = ctx.enter_context(tc.tile_pool(name="psA", bufs=2, space="PSUM"))
        psS = ctx.enter_context(tc.tile_pool(name="psS", bufs=3, space="PSUM"))
        psV = ctx.enter_context(tc.tile_pool(name="psV", bufs=2, space="PSUM"))

        # ---- persistent loads (host pre-arranged; all contiguous DMAs) ----
        # xA first so layer-0 V can start immediately; ow last (layer-6 only)
        xA = bigx.tile([P, KO, XW], bf16, tag="xA")
        nc.sync.dma_start(xA, d['xT'][:])
        wq_sb = wpool.tile([P, KO, D], bf16, tag="wq")
        wk_sb = wpool.tile([P, KO, D], bf16, tag="wk")
        wv_sb = wpool.tile([P, KO, D], bf16, tag="wv")
        fc1_sb = wpool.tile([P, KO, HD], bf16, tag="fc1")
        fc2_sb = wpool.tile([P, HC, D], bf16, tag="fc2")
        for sb, key in ((wv_sb, 'wvT'), (wq_sb, 'wqT'), (wk_sb, 'wkT'),
                        (fc1_sb, 'fc1T'), (fc2_sb, 'fc2T')):
            nc.sync.dma_start(sb, d[key][:])
        bq_sb = wpool.tile([P, KO], f32, tag="bq")
        bk_sb = wpool.tile([P, KO], f32, tag="bk")
        nc.sync.dma_start(bq_sb, d['bq'][:])
        nc.sync.dma_start(bk_sb, d['bk'][:])
        bv_bc = wpool.tile([P, D], f32, tag="bv")
        nc.gpsimd.dma_start(out=bv_bc, in_=bcast_ap(d['bv']))
        bv1k_sb = wpool.tile([1, D], f32, tag="bv1k")
        nc.sync.dma_start(bv1k_sb, d['bv1k'][:])
        fc1b_sb = wpool.tile([P, HC], f32, tag="fc1b")
        nc.sync.dma_start(fc1b_sb, d['fc1b'][:])
        fc2b_sb = wpool.tile([1, D], bf16, tag="fc2b")
        nc.sync.dma_start(fc2b_sb, d['fc2b'][:])
        mask_sb = wpool.tile([P, 1280], bf16, tag="mask")
        nc.sync.dma_start(mask_sb, d['mask'][:])
        ow_sb = wpool.tile([P, C, 8, D], bf16, tag="ow")
        nc.gpsimd.dma_start(out=ow_sb, in_=d['owT'][:])
        if affine:
            g_bc = wpool.tile([P, D], f32, tag="g")
            b_bc = wpool.tile([P, D], f32, tag="b")
            nc.gpsimd.dma_start(out=g_bc, in_=bcast_ap(d['lng']))
            nc.gpsimd.dma_start(out=b_bc, in_=bcast_ap(d['lnb']))

        ident = wpool.tile([P, P], bf16, tag="id")
        make_identity(nc, ident)
        ones_col = wpool.tile([P, 1], f32, tag="onc")
        nc.vector.memset(ones_col, 1.0)
        ones1 = wpool.tile([1, P], bf16, tag="on1")
        nc.vector.memset(ones1, 1.0)
        eps_sb = wpool.tile([P, 1], f32, tag="eps")
        nc.vector.memset(eps_sb, EPS)
        # V-totals row [1, 2*(4*65)]; ones-slots hold S (set once)
        vtot_sb = wpool.tile([1, 520], f32, tag="vtot")
        nc.vector.memset(
            vtot_sb.rearrange("p (g i a) -> p g i a", g=2, a=65)[:, :, :, 64:65],
            float(S))
        vtb = wpool.tile([P, 520], f32, tag="vtb")
        red = wpool.tile([P, C, 8], f32, tag="red")
        osb = wpool.tile([1, C], f32, tag="osb")
        scr = wpool.tile([P, D], bf16, tag="scr")
        scr2 = wpool.tile([P, D], bf16, tag="scr2")

        # mask views
        m_int = mask_sb[:, 0:256]
        m_e0 = mask_sb[:, 256:512]
        m_e8 = mask_sb[:, 512:768]
        ntri_lo = mask_sb[:, 768:896]
        ntri_lo_e = mask_sb[:, 896:1024]
        ntri_up = mask_sb[:, 1024:1152]
        ntri_up_e = mask_sb[:, 1152:1280]

        # x ping-pong tiles (padded, feature-major)
        xB = bigx.tile([P, KO, XW], bf16, tag="xB")
        nc.gpsimd.memset(xB[:, :, 0:64], 0.0)
        nc.gpsimd.memset(xB[:, :, 64 + S:XW], 0.0)
        x1T = bigx.tile([P, KO, S], bf16, tag="x1T")

        q_pad = [qkp.tile([P, QW], bf16, tag=f"q{mc}", name=f"q{mc}")
                 for mc in range(KO)]
        k_pad = [qkp.tile([P, XW], bf16, tag=f"k{mc}", name=f"k{mc}")
                 for mc in range(KO)]
        for mc in range(KO):
            nc.gpsimd.memset(q_pad[mc][:, 0:128], 0.0)
            nc.gpsimd.memset(q_pad[mc][:, 128 + S:QW], 0.0)
            nc.gpsimd.memset(k_pad[mc][:, 0:64], 0.0)
            nc.gpsimd.memset(k_pad[mc][:, 64 + S:XW], 0.0)
        va = vap.tile([P, 9, 520], bf16, tag="va")
        nc.vector.memset(
            va.rearrange("p s (i a) -> p s i a", a=65)[:, :, :, 64:65], 1.0)

        def layer_norm_to(src_ap, out_tile):
            """LayerNorm src [P,512] -> out_tile [P,512]."""
            st = small.tile([P, 6], f32, tag="st")
            mv = small.tile([P, 2], f32, tag="mv")
            nc.vector.bn_stats(out=st, in_=src_ap)
            nc.vector.bn_aggr(out=mv, in_=st)
            rstd = small.tile([P, 1], f32, tag="rs")
            nc.scalar.activation(out=rstd, in_=mv[:, 1:2], func=AF.Sqrt,
                                 bias=eps_sb[:, 0:1])
            nc.vector.reciprocal(out=rstd, in_=rstd)
            nc.vector.tensor_scalar(out=out_tile, in0=src_ap,
                                    scalar1=mv[:, 0:1], scalar2=rstd,
                                    op0=OP.subtract, op1=OP.mult)
            if affine:
                nc.vector.tensor_tensor(out=out_tile, in0=out_tile, in1=g_bc,
                                        op=OP.mult)
                nc.vector.tensor_tensor(out=out_tile, in0=out_tile, in1=b_bc,
                                        op=OP.add)

        xT = xA
        for L in range(ENC):
            xN = xB if (L % 2 == 0) else xA
            last = (L == ENC - 1)

            # ---------- V projection, 64-shifted token blocks ----------
            # xs reduce for V-totals is split per-ko and interleaved so the
            # DVE never blocks the V bias-adds for long.
            xs32 = small.tile([P, KO], f32, tag="xs")
            for stb in range(9):
                pv = psA.tile([P, 512], f32, tag="pj")
                for ko in range(KO):
                    nc.tensor.matmul(
                        pv, lhsT=xT[:, ko, 128 * stb:128 * stb + 128],
                        rhs=wv_sb[:, ko, :],
                        start=(ko == 0), stop=(ko == KO - 1))
                nc.vector.tensor_tensor(
                    out=va.rearrange("p s (g i a) -> p s g i a",
                                     g=2, a=65)[:, stb, :, :, 0:64],
                    in0=pv.rearrange("p (g i a) -> p g i a", g=2, a=64),
                    in1=bv_bc.rearrange("p (g i a) -> p g i a", g=2, a=64),
                    op=OP.add)
                if stb < KO:
                    nc.vector.reduce_sum(out=xs32[:, stb:stb + 1],
                                         in_=xT[:, stb:stb + 1, :],
                                         axis=mybir.AxisListType.X)

            # ---------- Q/K projections (feature-major, bf16, padded) ----------
            for mc in range(KO if STAGE >= 2 else 0):
                for half in range(2):
                    cs = slice(64 + half * 512, 64 + (half + 1) * 512)
                    pq = psA.tile([P, 512], f32, tag="pj")
                    for ko in range(KO):
                        nc.tensor.matmul(
                            pq, lhsT=wq_sb[:, ko, mc * P:(mc + 1) * P],
                            rhs=xT[:, ko, cs],
                            start=(ko == 0), stop=(ko == KO - 1))
                    nc.scalar.activation(
                        out=q_pad[mc][:, 128 + half * 512:128 + (half + 1) * 512],
                        in_=pq, func=AF.Identity, bias=bq_sb[:, mc:mc + 1])
                    pk = psA.tile([P, 512], f32, tag="pj")
                    for ko in range(KO):
                        nc.tensor.matmul(
                            pk, lhsT=wk_sb[:, ko, mc * P:(mc + 1) * P],
                            rhs=xT[:, ko, cs],
                            start=(ko == 0), stop=(ko == KO - 1))
                    nc.scalar.activation(
                        out=k_pad[mc][:, cs],
                        in_=pk, func=AF.Identity, bias=bk_sb[:, mc:mc + 1])

            # ---------- V totals: (sum_t x) @ wv + S*bv (f32) ----------
            xsr = small.tile([P, KO], bf16, tag="xsr")
            nc.scalar.copy(out=xsr, in_=xs32)
            pvt = psA.tile([P, 512], f32, tag="pj")
            for ko in range(KO):
                nc.tensor.matmul(
                    pvt[0:1, :], lhsT=xsr[:, ko:ko + 1],
                    rhs=wv_sb[:, ko, :],
                    start=(ko == 0), stop=(ko == KO - 1))
            nc.vector.tensor_tensor(
                out=vtot_sb.rearrange("p (g i a) -> p g i a",
                                      g=2, a=65)[:, :, :, 0:64],
                in0=pvt[0:1, :].rearrange("p (g i a) -> p g i a", g=2, a=64),
                in1=bv1k_sb.rearrange("p (g i a) -> p g i a", g=2, a=64),
                op=OP.add)
            nc.gpsimd.partition_broadcast(out_ap=vtb[:, :], in_ap=vtot_sb[0:1, :])

            # ---------- scores + probs (qb-centric key blocks) ----------
            # pc tiles per head-pair hp: [P, 512] = two heads x 256 query cols
            pcs = {}   # (hp, j) -> tile
            a_cur = None

            def attn_unit(j):
                """Scores/exp/mask for key-block j, all 8 heads."""
                mk = m_int if 0 < j < 8 else (m_e0 if j == 0 else m_e8)
                for h in range(H):
                    hp, sub = h // 2, h % 2
                    hr = slice(64 * sub, 64 * sub + 64)
                    sc = psS.tile([P, 256], f32, tag="s")
                    nc.tensor.matmul(
                        sc, lhsT=k_pad[hp][hr, 128 * j:128 * j + 128],
                        rhs=q_pad[hp][hr, 128 * j:128 * j + 256],
                        start=True, stop=True)
                    pc = pcp.tile([P, 256], bf16, tag=f"pc{h}", name=f"pc{h}")
                    nc.scalar.activation(out=pc, in_=sc, func=AF.Exp, scale=SCALE)
                    nc.vector.tensor_tensor(out=pc, in0=pc, in1=mk, op=OP.mult)
                    pcs[(h, j)] = pc

            def attn_av(qb):
                """AV + normalize for query block qb -> a_tok."""
                a_tok = atp.tile([P, D], f32, tag="at")
                for g in range(2):  # head group: heads 4g..4g+3
                    pav = psV.tile([P, 260], f32, tag="av")
                    nlo = ntri_lo_e if qb == 0 else ntri_lo
                    nup = ntri_up_e if qb == 7 else ntri_up
                    nc.tensor.matmul(pav, lhsT=nlo,
                                     rhs=va[:, qb, g * 260:(g + 1) * 260],
                                     start=True, stop=False)
                    nc.tensor.matmul(pav, lhsT=nup,
                                     rhs=va[:, qb + 1, g * 260:(g + 1) * 260],
                                     start=False, stop=False)
                    for i in range(4):
                        h = 4 * g + i
                        po = pav[:, i * 65:i * 65 + 65]
                        nc.tensor.matmul(
                            po, lhsT=pcs[(h, qb)][:, 128:256],
                            rhs=va[:, qb, g * 260 + i * 65:g * 260 + i * 65 + 65],
                            start=False, stop=False)
                        nc.tensor.matmul(
                            po, lhsT=pcs[(h, qb + 1)][:, 0:128],
                            rhs=va[:, qb + 1, g * 260 + i * 65:g * 260 + i * 65 + 65],
                            start=False, stop=(i == 3))
                    rc4 = small.tile([P, 4], f32, tag="rc")
                    nc.vector.tensor_scalar_add(
                        out=rc4.rearrange("p (i o) -> p i o", o=1),
                        in0=pav.rearrange("p (i a) -> p i a", a=65)[:, :, 64:65],
                        scalar1=float(S))
                    nc.vector.reciprocal(out=rc4, in_=rc4)
                    asl = a_tok[:, g * 256:(g + 1) * 256]
                    nc.vector.tensor_tensor(
                        out=asl.rearrange("p (i a) -> p i a", a=64),
                        in0=pav.rearrange("p (i a) -> p i a", a=65)[:, :, 0:64],
                        in1=vtb[:, g * 260:(g + 1) * 260].rearrange(
                            "p (i a) -> p i a", a=65)[:, :, 0:64],
                        op=OP.add)
                    nc.vector.tensor_tensor(
                        out=asl.rearrange("p (i a) -> p i a", a=64),
                        in0=asl.rearrange("p (i a) -> p i a", a=64),
                        in1=rep_last(rc4, 64), op=OP.mult)
                return a_tok

            def attn_ln(qb, a_tok):
                """LN1 -> xn (kept for residual) -> x1T feature-major."""
                xn = xnp.tile([P, D], bf16, tag=f"xn{qb}", name=f"xnt{qb}")
                layer_norm_to(a_tok, xn)
                pt = psA.tile([P, 512], bf16, tag="pt", bufs=1)
                for dc in range(KO):
                    nc.tensor.transpose(pt[:, dc * P:(dc + 1) * P],
                                        xn[:, dc * P:(dc + 1) * P], ident)
                nc.scalar.copy(
                    out=x1T[:, :, qb * P:(qb + 1) * P],
                    in_=pt.rearrange("p (ko t) -> p ko t", t=P))
                return xn

            if STAGE <= 2:
                continue
            xns = [None] * 8
            atoks = [None] * 8
            attn_unit(0)
            if STAGE >= 4:
                attn_unit(1)
                if STAGE >= 5:
                    atoks[0] = attn_av(0)
                for j in range(2, 9):
                    attn_unit(j)
                    if STAGE >= 5:
                        atoks[j - 1] = attn_av(j - 1)
                        xns[j - 2] = attn_ln(j - 2, atoks[j - 2])
                if STAGE == 5:
                    xns[7] = attn_ln(7, atoks[7])
            if STAGE <= 5:
                continue

            # ---------- FFN + residual + LN2 ----------
            hts = [htp.tile([P, 512], bf16, tag=f"h{hc}", name=f"h{hc}")
                   for hc in range(HC)]
            pend = None

            def emit_xpose(tb, xo):
                pt = psA.tile([P, 512], bf16, tag="pt", bufs=1)
                for dc in range(KO):
                    nc.tensor.transpose(pt[:, dc * P:(dc + 1) * P],
                                        xo[:, dc * P:(dc + 1) * P], ident)
                nc.scalar.copy(
                    out=xN[:, :, 64 + tb * P:64 + (tb + 1) * P],
                    in_=pt.rearrange("p (ko t) -> p ko t", t=P))

            def ffn_fc1(half):
                qs = slice(half * 512, (half + 1) * 512)
                for hc in range(HC):
                    ph = psA.tile([P, 512], f32, tag="pj")
                    for ko in range(KO):
                        nc.tensor.matmul(
                            ph, lhsT=fc1_sb[:, ko, hc * P:(hc + 1) * P],
                            rhs=x1T[:, ko, qs],
                            start=(ko == 0), stop=(ko == KO - 1))
                    nc.scalar.activation(out=hts[hc], in_=ph, func=AF.Relu,
                                         bias=fc1b_sb[:, hc:hc + 1])

            for half in range(2):
                ffn_fc1(half)
                if half == 0:
                    # qb7's LN1/transposes land while fc1-half0 runs on PE
                    xns[7] = attn_ln(7, atoks[7])
                for tb2 in range(4):
                    tb = half * 4 + tb2
                    pf = psA.tile([P, 512], f32, tag="pj")
                    # fc2 bias via K=1 matmul (start=True zeroes the bank)
                    nc.tensor.matmul(pf, lhsT=ones1[0:1, :], rhs=fc2b_sb[0:1, :],
                                     start=True, stop=False)
                    for hc in range(HC):
                        nc.tensor.matmul(
                            pf, lhsT=hts[hc][:, tb2 * P:(tb2 + 1) * P],
                            rhs=fc2_sb[:, hc, :],
                            start=False, stop=False)
                    # residual via identity matmul
                    nc.tensor.matmul(pf, lhsT=ident, rhs=xns[tb],
                                     start=False, stop=True)
                    xo = xxp.tile([P, D], bf16, tag="xo")
                    layer_norm_to(pf, xo)
                    if last:
                        # final projection partial: red[p, c, tb]
                        # (reduce split DVE/Act to avoid a DVE backlog tail)
                        for c in range(C):
                            sc_t = scr if c % 2 == 0 else scr2
                            nc.vector.tensor_tensor(
                                out=sc_t, in0=xo, in1=ow_sb[:, c, tb, :],
                                op=OP.mult)
                            if c % 2 == 0:
                                nc.vector.reduce_sum(
                                    out=red[:, c, tb:tb + 1], in_=sc_t,
                                    axis=mybir.AxisListType.X)
                            else:
                                nc.scalar.activation(
                                    out=sc_t, in_=sc_t, func=AF.Identity,
                                    accum_out=red[:, c, tb:tb + 1])
                    else:
                        # defer transpose one tb so PE never waits on LN2
                        if pend is not None:
                            emit_xpose(*pend)
                        pend = (tb, xo)
            if pend is not None:
                emit_xpose(*pend)
            xT = xN

        # ---------- final cross-partition reduce ----------
        if STAGE <= 8:
            nc.vector.memset(red[:, :, :], 0.0)
        pout = psS.tile([P, 256], f32, tag="s")
        nc.tensor.matmul(pout[0:1, 0:C * 8], lhsT=ones_col[:, 0:1],
                         rhs=red.rearrange("p c t -> p (c t)"),
                         start=True, stop=True)
        nc.vector.reduce_sum(
            out=osb, in_=pout[0:1, 0:C * 8].rearrange("p (c t) -> p c t", t=8),
            axis=mybir.AxisListType.X)
        nc.sync.dma_start(out_d[:], osb)

    nc.compile()
    return nc


def _prep(inputs):
    """Host-side input prep shared across cores. Returns (common, per_core, affine)."""
    import ml_dtypes
    bf = ml_dtypes.bfloat16

    emb = np.asarray(inputs['emb'], dtype=np.float32)
    idx = np.asarray(inputs['inputs'])
    pos = np.arange(S, dtype=np.float32)[:, None]
    div = np.exp(-np.log(10000.0) * np.arange(0, D, 2, dtype=np.float32) / D)
    ang = pos * div
    pe = np.zeros((S, D), dtype=np.float32)
    pe[:, 0::2] = np.sin(ang)
    pe[:, 1::2] = np.cos(ang)
    x0 = emb[idx] + pe[None]  # [B, S, D]

    # masks (bf16): interior M[p,c] = (p < c) & (p >= c-128) for key block
    # B_j vs query cols [128j-128, 128j+128)
    p_ = np.arange(P)[:, None]
    c_ = np.arange(256)[None, :]
    m_int = ((p_ < c_) & (p_ >= c_ - 128)).astype(np.float32)
    m_e0 = m_int * (p_ >= 64)     # j=0: keys [-64, 64), first 64 partitions fake
    m_e8 = m_int * (p_ < 64)      # j=8: keys [960, 1088), last 64 fake
    c128 = np.arange(128)[None, :]
    tri_lo = (p_ >= c128).astype(np.float32)
    tri_up = (p_ < c128).astype(np.float32)
    ntri_lo = -tri_lo
    ntri_lo_e = -(tri_lo * (p_ >= 64))
    ntri_up = -tri_up
    ntri_up_e = -(tri_up * (p_ < 64))
    mask = np.concatenate(
        [m_int, m_e0, m_e8, ntri_lo, ntri_lo_e, ntri_up, ntri_up_e],
        axis=1).astype(bf)

    ln_g = np.asarray(inputs['ln_g'], dtype=np.float32)
    ln_b = np.asarray(inputs['ln_b'], dtype=np.float32)
    affine = not (np.all(ln_g == 1.0) and np.all(ln_b == 0.0))

    def wmaj(wT, ko):  # [D, N] feature-major -> [P, ko, N]
        N = wT.shape[1]
        return np.ascontiguousarray(
            wT.reshape(ko, P, N).transpose(1, 0, 2)).astype(bf)

    out_w = np.asarray(inputs['out_w'], dtype=np.float32)
    owT = np.ascontiguousarray(
        out_w.reshape(C, 8, P, D).transpose(2, 0, 1, 3)).astype(bf)  # [P,C,8,D]

    bq = np.asarray(inputs['bq'], np.float32)
    bk = np.asarray(inputs['bk'], np.float32)
    bv = np.asarray(inputs['bv'], np.float32)
    fc1b = np.asarray(inputs['fc1_b'], np.float32)

    common = {
        'wqT': wmaj(np.asarray(inputs['wq'], np.float32).T, KO),
        'wkT': wmaj(np.asarray(inputs['wk'], np.float32).T, KO),
        'wvT': wmaj(np.asarray(inputs['wv'], np.float32).T, KO),
        'fc1T': wmaj(np.asarray(inputs['fc1_w'], np.float32).T, KO),
        'fc2T': wmaj(np.asarray(inputs['fc2_w'], np.float32).T, HC),
        'bq': np.ascontiguousarray(bq.reshape(KO, P).T),
        'bk': np.ascontiguousarray(bk.reshape(KO, P).T),
        'bv': np.ascontiguousarray(bv),
        'bv1k': np.ascontiguousarray(bv[None, :] * float(S)),
        'fc1b': np.ascontiguousarray(fc1b.reshape(HC, P).T),
        'fc2b': np.ascontiguousarray(
            np.asarray(inputs['fc2_b'], np.float32)[None, :]).astype(bf),
        'mask': mask,
        'owT': owT,
    }
    if affine:
        common['lng'] = np.ascontiguousarray(ln_g)
        common['lnb'] = np.ascontiguousarray(ln_b)
    per_core = []
    for b in range(B):
        xp = np.zeros((D, XW), dtype=np.float32)
        xp[:, 64:64 + S] = x0[b].T
        per_core.append({'xT': np.ascontiguousarray(
            xp.reshape(KO, P, XW).transpose(1, 0, 2)).astype(bf)})
    return common, per_core, affine


def kernel(**inputs):
    global LAST_EXEC_NS, LAST_RESULTS
    from concourse.bass_utils import run_bass_kernel_spmd

    common, per_core, affine = _prep(inputs)
    if affine not in _CACHE:
        _CACHE[affine] = _build(affine)
    nc = _CACHE[affine]

    in_maps = [dict(common, **pc) for pc in per_core]
    res = run_bass_kernel_spmd(nc, in_maps, list(range(B)), trace=TRACE)
    LAST_EXEC_NS = res.exec_time_ns
    LAST_RESULTS = res
    out = np.stack([res.results[b]["out"][0] for b in range(B)], axis=0)
    out = out + np.asarray(inputs['out_b'], np.float32)[None, :]
    return out.astype(np.float32)


# revision 44
# speedup vs baseline: 1.0015x; 1.0015x over previous
"""Trainium2 Bass kernel for nn_LocalModel (6-encoder local-attention transformer).

Sharding: data-parallel over batch - B=8 batch elements, one per NeuronCore.
Each core runs the full 6-layer encoder stack + final projection for its
batch element entirely on-chip (all weights resident in SBUF in bf16),
returning a [6]-vector; the host gathers them into the [8, 6] output.

Attention uses the zero-masked-softmax identity: with out-of-window scores
set to 0 (not -inf), softmax over the full sequence satisfies
    out_i = (sum_{j in W} (e^{s_ij} - 1) v_j + sum_all v_j)
          / (sum_{j in W} (e^{s_ij} - 1) + S)
The banded scores are computed qb-centric: key blocks B_j = [128j-64,
128j+64) (tokens padded by 64 zeros each side) against query cols
[128j-128, 128j+128), giving uniform triangular masks. The "-1" term is
folded into the PSUM accumulation via negative-mask matmuls against a
64-shifted V copy (va_shift), so the DVE only does exp-mask multiply.
"""
import sys
import numpy as np

sys.path.insert(0, "/opt/trn_rl_repo")

B, S, D = 8, 1024, 512
H, Dh, W = 8, 64, 64
HD = 2048           # ffn hidden
C = 6               # classes
ENC = 6
EPS = 1e-5
P = 128
KO = D // P         # 4
HC = HD // P        # 16
SCALE = Dh ** -0.5
XW = 64 + S + 64    # padded token width for x / k tiles (1152)
QW = 128 + S + 128  # padded token width for q tiles (1280)

_CACHE = {}
LAST_EXEC_NS = None
LAST_RESULTS = None
TRACE = False


def _build(affine: bool):
    import os
    STAGE = int(os.environ.get("KSTAGE", "9"))
    import concourse.bass as bass
    import concourse.tile as tile
    from concourse import bacc, mybir
    from concourse.masks import make_identity

    f32 = mybir.dt.float32
    bf16 = mybir.dt.bfloat16
    f16 = mybir.dt.float16
    AF = mybir.ActivationFunctionType
    OP = mybir.AluOpType

    nc = bacc.Bacc()
    d = {}
    d['xT'] = nc.declare_dram_parameter("xT", [P, KO, XW], bf16, isOutput=False)
    for w in ("wqT", "wkT", "wvT"):
        d[w] = nc.declare_dram_parameter(w, [P, KO, D], bf16, isOutput=False)
    d['fc1T'] = nc.declare_dram_parameter("fc1T", [P, KO, HD], bf16, isOutput=False)
    d['fc2T'] = nc.declare_dram_parameter("fc2T", [P, HC, D], bf16, isOutput=False)
    d['owT'] = nc.declare_dram_parameter("owT", [P, C, 8, D], bf16, isOutput=False)
    d['bq'] = nc.declare_dram_parameter("bq", [P, KO], f32, isOutput=False)
    d['bk'] = nc.declare_dram_parameter("bk", [P, KO], f32, isOutput=False)
    d['bv'] = nc.declare_dram_parameter("bv", [D], f32, isOutput=False)
    d['bv1k'] = nc.declare_dram_parameter("bv1k", [1, D], f32, isOutput=False)
    d['fc1b'] = nc.declare_dram_parameter("fc1b", [P, HC], f32, isOutput=False)
    d['fc2b'] = nc.declare_dram_parameter("fc2b", [1, D], bf16, isOutput=False)
    # masks: [m_int 256 | m_e0 256 | m_e8 256 | ntri_lo 128 | ntri_lo_e 128
    #         | ntri_up 128 | ntri_up_e 128]  (bf16)
    d['mask'] = nc.declare_dram_parameter("mask", [P, 1280], bf16, isOutput=False)
    if affine:
        d['lng'] = nc.declare_dram_parameter("lng", [D], f32, isOutput=False)
        d['lnb'] = nc.declare_dram_parameter("lnb", [D], f32, isOutput=False)
    out_d = nc.declare_dram_parameter("out", [1, C], f32, isOutput=True)

    def bcast_ap(dram_h, parts=P):
        # replicate a [N] dram vector across `parts` partitions
        a = dram_h[:]
        return bass.AP(tensor=a.tensor, offset=a.offset,
                       ap=[[0, parts]] + [list(x) for x in a.ap])

    def rep_mid(ap2d, reps):
        # [P, N] -> [P, reps, N] with stride-0 middle axis
        return bass.AP(tensor=ap2d.tensor, offset=ap2d.offset,
                       ap=[list(ap2d.ap[0]), [0, reps], list(ap2d.ap[1])])

    def rep_last(ap2d, reps):
        # [P, N] -> [P, N, reps] with stride-0 last axis
        return bass.AP(tensor=ap2d.tensor, offset=ap2d.offset,
                       ap=[list(ap2d.ap[0]), list(ap2d.ap[1]), [0, reps]])

    from contextlib import ExitStack
    with tile.TileContext(nc) as tc, ExitStack() as ctx:
        wpool = ctx.enter_context(tc.tile_pool(name="wpool", bufs=1))
        bigx = ctx.enter_context(tc.tile_pool(name="bigx", bufs=1))
        qkp = ctx.enter_context(tc.tile_pool(name="qkp", bufs=1))
        vap = ctx.enter_context(tc.tile_pool(name="vap", bufs=1))
        pcp = ctx.enter_context(tc.tile_pool(name="pcp", bufs=3))
        atp = ctx.enter_context(tc.tile_pool(name="atp", bufs=2))
        xnp = ctx.enter_context(tc.tile_pool(name="xnp", bufs=1))
        htp = ctx.enter_context(tc.tile_pool(name="htp", bufs=1))
        xxp = ctx.enter_context(tc.tile_pool(name="xxp", bufs=2))
        tmp = ctx.enter_context(tc.tile_pool(name="tmp", bufs=3))
        small = ctx.enter_context(tc.tile_pool(name="small", bufs=4))
        psA = ctx.enter_context(tc.tile_pool(name="psA", bufs=3, space="PSUM"))
        psS = ctx.enter_context(tc.tile_pool(name="psS", bufs=2, space="PSUM"))
        psV = ctx.enter_context(tc.tile_pool(name="psV", bufs=2, space="PSUM"))

        # ---- persistent loads (host pre-arranged; all contiguous DMAs) ----
        # xA first so layer-0 V can start immediately; ow last (layer-6 only)
        xA = bigx.tile([P, KO, XW], bf16, tag="xA")
        nc.sync.dma_start(xA, d['xT'][:])
        wq_sb = wpool.tile([P, KO, D], bf16, tag="wq")
        wk_sb = wpool.tile([P, KO, D], bf16, tag="wk")
        wv_sb = wpool.tile([P, KO, D], bf16, tag="wv")
        fc1_sb = wpool.tile([P, KO, HD], bf16, tag="fc1")
        fc2_sb = wpool.tile([P, HC, D], bf16, tag="fc2")
        for sb, key in ((wv_sb, 'wvT'), (wq_sb, 'wqT'), (wk_sb, 'wkT')):
            nc.sync.dma_start(sb, d[key][:])
        nc.scalar.dma_start(out=fc1_sb, in_=d['fc1T'][:])
        nc.scalar.dma_start(out=fc2_sb, in_=d['fc2T'][:])
        bq_sb = wpool.tile([P, KO], f32, tag="bq")
        bk_sb = wpool.tile([P, KO], f32, tag="bk")
        nc.sync.dma_start(bq_sb, d['bq'][:])
        nc.sync.dma_start(bk_sb, d['bk'][:])
        bv_bc = wpool.tile([P, D], f32, tag="bv")
        nc.gpsimd.dma_start(out=bv_bc, in_=bcast_ap(d['bv']))
        bv1k_sb = wpool.tile([1, D], f32, tag="bv1k")
        nc.sync.dma_start(bv1k_sb, d['bv1k'][:])
        fc1b_sb = wpool.tile([P, HC], f32, tag="fc1b")
        nc.sync.dma_start(fc1b_sb, d['fc1b'][:])
        fc2b_sb = wpool.tile([1, D], bf16, tag="fc2b")
        nc.sync.dma_start(fc2b_sb, d['fc2b'][:])
        mask_sb = wpool.tile([P, 1280], bf16, tag="mask")
        nc.sync.dma_start(mask_sb, d['mask'][:])
        ow_sb = wpool.tile([P, C, 8, D], bf16, tag="ow")
        nc.gpsimd.dma_start(out=ow_sb, in_=d['owT'][:])
        if affine:
            g_bc = wpool.tile([P, D], f32, tag="g")
            b_bc = wpool.tile([P, D], f32, tag="b")
            nc.gpsimd.dma_start(out=g_bc, in_=bcast_ap(d['lng']))
            nc.gpsimd.dma_start(out=b_bc, in_=bcast_ap(d['lnb']))

        ident = wpool.tile([P, P], bf16, tag="id")
        make_identity(nc, ident)
        ones_col = wpool.tile([P, 1], f32, tag="onc")
        nc.vector.memset(ones_col, 1.0)
        ones1 = wpool.tile([1, P], bf16, tag="on1")
        nc.vector.memset(ones1, 1.0)
        eps_sb = wpool.tile([P, 1], f32, tag="eps")
        nc.vector.memset(eps_sb, EPS)
        # V-totals row [1, 2*(4*65)]; ones-slots hold S (set once)
        vtot_sb = wpool.tile([1, 520], f32, tag="vtot")
        nc.vector.memset(
            vtot_sb.rearrange("p (g i a) -> p g i a", g=2, a=65)[:, :, :, 64:65],
            float(S))
        vtb = wpool.tile([P, 520], f32, tag="vtb")
        red = wpool.tile([P, C, 8], f32, tag="red")
        osb = wpool.tile([1, C], f32, tag="osb")
        scr = wpool.tile([P, D], bf16, tag="scr")
        scr2 = wpool.tile([P, D], bf16, tag="scr2")
        scr3 = wpool.tile([P, D], bf16, tag="scr3")

        # mask views
        m_int = mask_sb[:, 0:256]
        m_e0 = mask_sb[:, 256:512]
        m_e8 = mask_sb[:, 512:768]
        ntri_lo = mask_sb[:, 768:896]
        ntri_lo_e = mask_sb[:, 896:1024]
        ntri_up = mask_sb[:, 1024:1152]
        ntri_up_e = mask_sb[:, 1152:1280]

        # x ping-pong tiles (padded, feature-major)
        xB = bigx.tile([P, KO, XW], bf16, tag="xB")
        nc.gpsimd.memset(xB[:, :, 0:64], 0.0)
        nc.gpsimd.memset(xB[:, :, 64 + S:XW], 0.0)
        x1T = bigx.tile([P, 8, KO, P], bf16, tag="x1T")

        q_pad = [qkp.tile([P, QW], bf16, tag=f"q{mc}", name=f"q{mc}")
                 for mc in range(KO)]
        k_pad = [qkp.tile([P, XW], bf16, tag=f"k{mc}", name=f"k{mc}")
                 for mc in range(KO)]
        for mc in range(KO):
            nc.gpsimd.memset(q_pad[mc][:, 0:128], 0.0)
            nc.gpsimd.memset(q_pad[mc][:, 128 + S:QW], 0.0)
            nc.gpsimd.memset(k_pad[mc][:, 0:64], 0.0)
            nc.gpsimd.memset(k_pad[mc][:, 64 + S:XW], 0.0)
        va = vap.tile([P, 9, 520], bf16, tag="va")
        nc.vector.memset(
            va.rearrange("p s (i a) -> p s i a", a=65)[:, :, :, 64:65], 1.0)

        def layer_norm_to(src_ap, out_tile):
            """LayerNorm src [P,512] -> out_tile [P,512]."""
            st = small.tile([P, 6], f32, tag="st")
            mv = small.tile([P, 2], f32, tag="mv")
            nc.vector.bn_stats(out=st, in_=src_ap)
            nc.vector.bn_aggr(out=mv, in_=st)
            rstd = small.tile([P, 1], f32, tag="rs")
            nc.scalar.activation(out=rstd, in_=mv[:, 1:2], func=AF.Sqrt,
                                 bias=eps_sb[:, 0:1])
            nc.vector.reciprocal(out=rstd, in_=rstd)
            nc.vector.tensor_scalar(out=out_tile, in0=src_ap,
                                    scalar1=mv[:, 0:1], scalar2=rstd,
                                    op0=OP.subtract, op1=OP.mult)
            if affine:
                nc.vector.tensor_tensor(out=out_tile, in0=out_tile, in1=g_bc,
                                        op=OP.mult)
                nc.vector.tensor_tensor(out=out_tile, in0=out_tile, in1=b_bc,
                                        op=OP.add)

        xT = xA
        for L in range(ENC):
            xN = xB if (L % 2 == 0) else xA
            last = (L == ENC - 1)

            # ---------- V projection, 64-shifted token blocks ----------
            # xs reduce for V-totals is split per-ko and interleaved so the
            # DVE never blocks the V bias-adds for long.
            xs32 = small.tile([P, KO], f32, tag="xs")
            for stb in range(9):
                pv = psA.tile([P, 512], f32, tag="pj")
                for ko in range(KO):
                    nc.tensor.matmul(
                        pv, lhsT=xT[:, ko, 128 * stb:128 * stb + 128],
                        rhs=wv_sb[:, ko, :],
                        start=(ko == 0), stop=(ko == KO - 1))
                nc.vector.tensor_tensor(
                    out=va.rearrange("p s (g i a) -> p s g i a",
                                     g=2, a=65)[:, stb, :, :, 0:64],
                    in0=pv.rearrange("p (g i a) -> p g i a", g=2, a=64),
                    in1=bv_bc.rearrange("p (g i a) -> p g i a", g=2, a=64),
                    op=OP.add)
                if stb < KO:
                    nc.vector.reduce_sum(out=xs32[:, stb:stb + 1],
                                         in_=xT[:, stb:stb + 1, :],
                                         axis=mybir.AxisListType.X)

            # ---------- Q/K projections (feature-major, bf16, padded) ----------
            for mc in range(KO if STAGE >= 2 else 0):
                for half in range(2):
                    cs = slice(64 + half * 512, 64 + (half + 1) * 512)
                    pq = psA.tile([P, 512], f32, tag="pj")
                    for ko in range(KO):
                        nc.tensor.matmul(
                            pq, lhsT=wq_sb[:, ko, mc * P:(mc + 1) * P],
                            rhs=xT[:, ko, cs],
                            start=(ko == 0), stop=(ko == KO - 1))
                    nc.scalar.activation(
                        out=q_pad[mc][:, 128 + half * 512:128 + (half + 1) * 512],
                        in_=pq, func=AF.Identity, bias=bq_sb[:, mc:mc + 1])
                    pk = psA.tile([P, 512], f32, tag="pj")
                    for ko in range(KO):
                        nc.tensor.matmul(
                            pk, lhsT=wk_sb[:, ko, mc * P:(mc + 1) * P],
                            rhs=xT[:, ko, cs],
                            start=(ko == 0), stop=(ko == KO - 1))
                    nc.scalar.activation(
                        out=k_pad[mc][:, cs],
                        in_=pk, func=AF.Identity, bias=bk_sb[:, mc:mc + 1])

            # ---------- V totals: (sum_t x) @ wv + S*bv (f32) ----------
            xsr = small.tile([P, KO], bf16, tag="xsr")
            nc.scalar.copy(out=xsr, in_=xs32)
            pvt = psA.tile([P, 512], f32, tag="pj")
            for ko in range(KO):
                nc.tensor.matmul(
                    pvt[0:1, :], lhsT=xsr[:, ko:ko + 1],
                    rhs=wv_sb[:, ko, :],
                    start=(ko == 0), stop=(ko == KO - 1))
            nc.vector.tensor_tensor(
                out=vtot_sb.rearrange("p (g i a) -> p g i a",
                                      g=2, a=65)[:, :, :, 0:64],
                in0=pvt[0:1, :].rearrange("p (g i a) -> p g i a", g=2, a=64),
                in1=bv1k_sb.rearrange("p (g i a) -> p g i a", g=2, a=64),
                op=OP.add)
            nc.gpsimd.partition_broadcast(out_ap=vtb[:, :], in_ap=vtot_sb[0:1, :])

            # ---------- scores + probs (qb-centric key blocks) ----------
            # pc tiles per head-pair hp: [P, 512] = two heads x 256 query cols
            pcs = {}   # (hp, j) -> tile
            a_cur = None

            def attn_part(j, part):
                """Scores/exp/mask for key-block j, heads 4*part..4*part+3."""
                mk = m_int if 0 < j < 8 else (m_e0 if j == 0 else m_e8)
                for h in range(4 * part, 4 * part + 4):
                    hp, sub = h // 2, h % 2
                    hr = slice(64 * sub, 64 * sub + 64)
                    sc = psS.tile([P, 256], f32, tag="s")
                    nc.tensor.matmul(
                        sc, lhsT=k_pad[hp][hr, 128 * j:128 * j + 128],
                        rhs=q_pad[hp][hr, 128 * j:128 * j + 256],
                        start=True, stop=True)
                    pc = pcp.tile([P, 256], bf16, tag=f"pc{h}", name=f"pc{h}")
                    nc.scalar.activation(out=pc, in_=sc, func=AF.Exp, scale=SCALE)
                    nc.vector.tensor_tensor(out=pc, in0=pc, in1=mk, op=OP.mult)
                    pcs[(h, j)] = pc

            def attn_unit(j):
                attn_part(j, 0)
                attn_part(j, 1)

            def attn_av(qb, unit_j=None):
                """AV + normalize for query block qb -> a_tok.
                Interleaves score emission for key-block unit_j so the PE
                always has AV work while Act/DVE produce the next probs."""
                a_tok = atp.tile([P, D], f32, tag="at")
                for g in range(2):  # head group: heads 4g..4g+3
                    if unit_j is not None:
                        attn_part(unit_j, g)
                    pav = psV.tile([P, 260], f32, tag="av")
                    nlo = ntri_lo_e if qb == 0 else ntri_lo
                    nup = ntri_up_e if qb == 7 else ntri_up
                    nc.tensor.matmul(pav, lhsT=nlo,
                                     rhs=va[:, qb, g * 260:(g + 1) * 260],
                                     start=True, stop=False)
                    nc.tensor.matmul(pav, lhsT=nup,
                                     rhs=va[:, qb + 1, g * 260:(g + 1) * 260],
                                     start=False, stop=False)
                    for i in range(4):
                        h = 4 * g + i
                        po = pav[:, i * 65:i * 65 + 65]
                        nc.tensor.matmul(
                            po, lhsT=pcs[(h, qb)][:, 128:256],
                            rhs=va[:, qb, g * 260 + i * 65:g * 260 + i * 65 + 65],
                            start=False, stop=False)
                        nc.tensor.matmul(
                            po, lhsT=pcs[(h, qb + 1)][:, 0:128],
                            rhs=va[:, qb + 1, g * 260 + i * 65:g * 260 + i * 65 + 65],
                            start=False, stop=(i == 3))
                    rc4 = small.tile([P, 4], f32, tag="rc")
                    nc.vector.tensor_scalar_add(
                        out=rc4.rearrange("p (i o) -> p i o", o=1),
                        in0=pav.rearrange("p (i a) -> p i a", a=65)[:, :, 64:65],
                        scalar1=float(S))
                    nc.vector.reciprocal(out=rc4, in_=rc4)
                    asl = a_tok[:, g * 256:(g + 1) * 256]
                    nc.vector.tensor_tensor(
                        out=asl.rearrange("p (i a) -> p i a", a=64),
                        in0=pav.rearrange("p (i a) -> p i a", a=65)[:, :, 0:64],
                        in1=vtb[:, g * 260:(g + 1) * 260].rearrange(
                            "p (i a) -> p i a", a=65)[:, :, 0:64],
                        op=OP.add)
                    nc.vector.tensor_tensor(
                        out=asl.rearrange("p (i a) -> p i a", a=64),
                        in0=asl.rearrange("p (i a) -> p i a", a=64),
                        in1=rep_last(rc4, 64), op=OP.mult)
                return a_tok

            def attn_ln(qb, a_tok):
                """LN1 -> xn (kept for residual) -> x1T feature-major."""
                xn = xnp.tile([P, D], bf16, tag=f"xn{qb}", name=f"xnt{qb}")
                layer_norm_to(a_tok, xn)
                pt = psA.tile([P, 512], bf16, tag="pt", bufs=1)
                for dc in range(KO):
                    nc.tensor.transpose(pt[:, dc * P:(dc + 1) * P],
                                        xn[:, dc * P:(dc + 1) * P], ident)
                nc.scalar.copy(
                    out=x1T[:, qb, :, :],
                    in_=pt.rearrange("p (ko t) -> p ko t", t=P))
                return xn

            if STAGE <= 2:
                continue
            xns = [None] * 8
            atoks = [None] * 8
            attn_unit(0)
            if STAGE >= 4:
                attn_unit(1)
                if STAGE >= 5:
                    atoks[0] = attn_av(0, unit_j=2)
                    for j in range(3, 9):
                        atoks[j - 2] = attn_av(j - 2, unit_j=j)
                        xns[j - 3] = attn_ln(j - 3, atoks[j - 3])
                    atoks[7] = attn_av(7)
                    xns[6] = attn_ln(6, atoks[6])
                else:
                    for j in range(2, 9):
                        attn_unit(j)
            if STAGE <= 5:
                continue

            # ---------- FFN + residual + LN2 ----------
            hts = [htp.tile([P, 512], bf16, tag=f"h{hc}", name=f"h{hc}")
                   for hc in range(HC)]
            pend = None

            def emit_xpose(tb, xo):
                pt = psA.tile([P, 512], bf16, tag="pt", bufs=1)
                for dc in range(KO):
                    nc.tensor.transpose(pt[:, dc * P:(dc + 1) * P],
                                        xo[:, dc * P:(dc + 1) * P], ident)
                nc.scalar.copy(
                    out=xN[:, :, 64 + tb * P:64 + (tb + 1) * P],
                    in_=pt.rearrange("p (ko t) -> p ko t", t=P))

            def ffn_fc1(half):
                qbs = slice(half * 4, (half + 1) * 4)
                for hc in range(HC):
                    ph = psA.tile([P, 512], f32, tag="pj")
                    for ko in range(KO):
                        nc.tensor.matmul(
                            ph, lhsT=fc1_sb[:, ko, hc * P:(hc + 1) * P],
                            rhs=x1T[:, qbs, ko, :],
                            start=(ko == 0), stop=(ko == KO - 1))
                    nc.scalar.activation(out=hts[hc], in_=ph, func=AF.Relu,
                                         bias=fc1b_sb[:, hc:hc + 1])

            for half in range(2):
                ffn_fc1(half)
                if half == 0:
                    # qb7's LN1/transposes land while fc1-half0 runs on PE
                    xns[7] = attn_ln(7, atoks[7])
                for tb2 in range(4):
                    tb = half * 4 + tb2
                    pf = psA.tile([P, 512], f32, tag="pj")
                    # fc2 bias via K=1 matmul (start=True zeroes the bank)
                    nc.tensor.matmul(pf, lhsT=ones1[0:1, :], rhs=fc2b_sb[0:1, :],
                                     start=True, stop=False)
                    for hc in range(HC):
                        nc.tensor.matmul(
                            pf, lhsT=hts[hc][:, tb2 * P:(tb2 + 1) * P],
                            rhs=fc2_sb[:, hc, :],
                            start=False, stop=False)
                    # residual via identity matmul
                    nc.tensor.matmul(pf, lhsT=ident, rhs=xns[tb],
                                     start=False, stop=True)
                    xo = xxp.tile([P, D], bf16, tag="xo")
                    layer_norm_to(pf, xo)
                    if last:
                        # final projection partial: red[p, c, tb]
                        # (work split DVE/Act/GpSimd to avoid a backlog tail)
                        for c in range(C):
                            w = c % 3
                            sc_t = (scr, scr2, scr3)[w]
                            eng = nc.gpsimd if w == 2 else nc.vector
                            eng.tensor_tensor(
                                out=sc_t, in0=xo, in1=ow_sb[:, c, tb, :],
                                op=OP.mult)
                            if w == 0:
                                nc.vector.reduce_sum(
                                    out=red[:, c, tb:tb + 1], in_=sc_t,
                                    axis=mybir.AxisListType.X)
                            else:
                                nc.scalar.activation(
                                    out=sc_t, in_=sc_t, func=AF.Identity,
                                    accum_out=red[:, c, tb:tb + 1])
                    else:
                        # defer transpose one tb so PE never waits on LN2
                        if pend is not None:
                            emit_xpose(*pend)
                        pend = (tb, xo)
            if pend is not None:
                emit_xpose(*pend)
            xT = xN

        # ---------- final cross-partition reduce ----------
        if STAGE <= 8:
            nc.vector.memset(red[:, :, :], 0.0)
        pout = psS.tile([P, 256], f32, tag="s")
        nc.tensor.matmul(pout[0:1, 0:C * 8], lhsT=ones_col[:, 0:1],
                         rhs=red.rearrange("p c t -> p (c t)"),
                         start=True, stop=True)
        nc.vector.reduce_sum(
            out=osb, in_=pout[0:1, 0:C * 8].rearrange("p (c t) -> p c t", t=8),
            axis=mybir.AxisListType.X)
        nc.sync.dma_start(out_d[:], osb)

    nc.compile()
    return nc


def _prep(inputs):
    """Host-side input prep shared across cores. Returns (common, per_core, affine)."""
    import ml_dtypes
    bf = ml_dtypes.bfloat16

    emb = np.asarray(inputs['emb'], dtype=np.float32)
    idx = np.asarray(inputs['inputs'])
    pos = np.arange(S, dtype=np.float32)[:, None]
    div = np.exp(-np.log(10000.0) * np.arange(0, D, 2, dtype=np.float32) / D)
    ang = pos * div
    pe = np.zeros((S, D), dtype=np.float32)
    pe[:, 0::2] = np.sin(ang)
    pe[:, 1::2] = np.cos(ang)
    x0 = emb[idx] + pe[None]  # [B, S, D]

    # masks (bf16): interior M[p,c] = (p < c) & (p >= c-128) for key block
    # B_j vs query cols [128j-128, 128j+128)
    p_ = np.arange(P)[:, None]
    c_ = np.arange(256)[None, :]
    m_int = ((p_ < c_) & (p_ >= c_ - 128)).astype(np.float32)
    m_e0 = m_int * (p_ >= 64)     # j=0: keys [-64, 64), first 64 partitions fake
    m_e8 = m_int * (p_ < 64)      # j=8: keys [960, 1088), last 64 fake
    c128 = np.arange(128)[None, :]
    tri_lo = (p_ >= c128).astype(np.float32)
    tri_up = (p_ < c128).astype(np.float32)
    ntri_lo = -tri_lo
    ntri_lo_e = -(tri_lo * (p_ >= 64))
    ntri_up = -tri_up
    ntri_up_e = -(tri_up * (p_ < 64))
    mask = np.concatenate(
        [m_int, m_e0, m_e8, ntri_lo, ntri_lo_e, ntri_up, ntri_up_e],
        axis=1).astype(bf)

    ln_g = np.asarray(inputs['ln_g'], dtype=np.float32)
    ln_b = np.asarray(inputs['ln_b'], dtype=np.float32)
    affine = not (np.all(ln_g == 1.0) and np.all(ln_b == 0.0))

    def wmaj(wT, ko):  # [D, N] feature-major -> [P, ko, N]
        N = wT.shape[1]
        return np.ascontiguousarray(
            wT.reshape(ko, P, N).transpose(1, 0, 2)).astype(bf)

    out_w = np.asarray(inputs['out_w'], dtype=np.float32)
    owT = np.ascontiguousarray(
        out_w.reshape(C, 8, P, D).transpose(2, 0, 1, 3)).astype(bf)  # [P,C,8,D]

    bq = np.asarray(inputs['bq'], np.float32)
    bk = np.asarray(inputs['bk'], np.float32)
    bv = np.asarray(inputs['bv'], np.float32)
    fc1b = np.asarray(inputs['fc1_b'], np.float32)

    common = {
        'wqT': wmaj(np.asarray(inputs['wq'], np.float32).T, KO),
        'wkT': wmaj(np.asarray(inputs['wk'], np.float32).T, KO),
        'wvT': wmaj(np.asarray(inputs['wv'], np.float32).T, KO),
        'fc1T': wmaj(np.asarray(inputs['fc1_w'], np.float32).T, KO),
        'fc2T': wmaj(np.asarray(inputs['fc2_w'], np.float32).T, HC),
        'bq': np.ascontiguousarray(bq.reshape(KO, P).T),
        'bk': np.ascontiguousarray(bk.reshape(KO, P).T),
        'bv': np.ascontiguousarray(bv),
        'bv1k': np.ascontiguousarray(bv[None, :] * float(S)),
        'fc1b': np.ascontiguousarray(fc1b.reshape(HC, P).T),
        'fc2b': np.ascontiguousarray(
            np.asarray(inputs['fc2_b'], np.float32)[None, :]).astype(bf),
        'mask': mask,
        'owT': owT,
    }
    if affine:
        common['lng'] = np.ascontiguousarray(ln_g)
        common['lnb'] = np.ascontiguousarray(ln_b)
    per_core = []
    for b in range(B):
        xp = np.zeros((D, XW), dtype=np.float32)
        xp[:, 64:64 + S] = x0[b].T
        per_core.append({'xT': np.ascontiguousarray(
            xp.reshape(KO, P, XW).transpose(1, 0, 2)).astype(bf)})
    return common, per_core, affine


def kernel(**inputs):
    global LAST_EXEC_NS, LAST_RESULTS
    from concourse.bass_utils import run_bass_kernel_spmd

    common, per_core, affine = _prep(inputs)
    if affine not in _CACHE:
        _CACHE[affine] = _build(affine)
    nc = _CACHE[affine]

    in_maps = [dict(common, **pc) for pc in per_core]
    res = run_bass_kernel_spmd(nc, in_maps, list(range(B)), trace=TRACE)
    LAST_EXEC_NS = res.exec_time_ns
    LAST_RESULTS = res
    out = np.stack([res.results[b]["out"][0] for b in range(B)], axis=0)
    out = out + np.asarray(inputs['out_b'], np.float32)[None, :]
    return out.astype(np.float32)


# revision 45
# speedup vs baseline: 1.0052x; 1.0038x over previous
"""Trainium2 Bass kernel for nn_LocalModel (6-encoder local-attention transformer).

Sharding: data-parallel over batch - B=8 batch elements, one per NeuronCore.
Each core runs the full 6-layer encoder stack + final projection for its
batch element entirely on-chip (all weights resident in SBUF in bf16),
returning a [6]-vector; the host gathers them into the [8, 6] output.

Attention uses the zero-masked-softmax identity: with out-of-window scores
set to 0 (not -inf), softmax over the full sequence satisfies
    out_i = (sum_{j in W} (e^{s_ij} - 1) v_j + sum_all v_j)
          / (sum_{j in W} (e^{s_ij} - 1) + S)
The banded scores are computed qb-centric: key blocks B_j = [128j-64,
128j+64) (tokens padded by 64 zeros each side) against query cols
[128j-128, 128j+128), giving uniform triangular masks. The "-1" term is
folded into the PSUM accumulation via negative-mask matmuls against a
64-shifted V copy (va_shift), so the DVE only does exp-mask multiply.
"""
import sys
import numpy as np

sys.path.insert(0, "/opt/trn_rl_repo")

B, S, D = 8, 1024, 512
H, Dh, W = 8, 64, 64
HD = 2048           # ffn hidden
C = 6               # classes
ENC = 6
EPS = 1e-5
P = 128
KO = D // P         # 4
HC = HD // P        # 16
SCALE = Dh ** -0.5
XW = 64 + S + 64    # padded token width for x / k tiles (1152)
QW = 128 + S + 128  # padded token width for q tiles (1280)

_CACHE = {}
LAST_EXEC_NS = None
LAST_RESULTS = None
TRACE = False


def _build(affine: bool):
    import os
    STAGE = int(os.environ.get("KSTAGE", "9"))
    import concourse.bass as bass
    import concourse.tile as tile
    from concourse import bacc, mybir
    from concourse.masks import make_identity

    f32 = mybir.dt.float32
    bf16 = mybir.dt.bfloat16
    f16 = mybir.dt.float16
    AF = mybir.ActivationFunctionType
    OP = mybir.AluOpType

    nc = bacc.Bacc()
    d = {}
    d['xT'] = nc.declare_dram_parameter("xT", [P, KO, XW], bf16, isOutput=False)
    for w in ("wqT", "wkT", "wvT"):
        d[w] = nc.declare_dram_parameter(w, [P, KO, D], bf16, isOutput=False)
    d['fc1T'] = nc.declare_dram_parameter("fc1T", [P, KO, HD], bf16, isOutput=False)
    d['fc2T'] = nc.declare_dram_parameter("fc2T", [P, HC, D], bf16, isOutput=False)
    d['owT'] = nc.declare_dram_parameter("owT", [P, C, 8, D], bf16, isOutput=False)
    d['bq'] = nc.declare_dram_parameter("bq", [P, KO], f32, isOutput=False)
    d['bk'] = nc.declare_dram_parameter("bk", [P, KO], f32, isOutput=False)
    d['bv'] = nc.declare_dram_parameter("bv", [D], f32, isOutput=False)
    d['bv1k'] = nc.declare_dram_parameter("bv1k", [1, D], f32, isOutput=False)
    d['fc1b'] = nc.declare_dram_parameter("fc1b", [P, HC], f32, isOutput=False)
    d['fc2b'] = nc.declare_dram_parameter("fc2b", [1, D], bf16, isOutput=False)
    # masks: [m_int 256 | m_e0 256 | m_e8 256 | ntri_lo 128 | ntri_lo_e 128
    #         | ntri_up 128 | ntri_up_e 128]  (bf16)
    d['mask'] = nc.declare_dram_parameter("mask", [P, 1280], bf16, isOutput=False)
    if affine:
        d['lng'] = nc.declare_dram_parameter("lng", [D], f32, isOutput=False)
        d['lnb'] = nc.declare_dram_parameter("lnb", [D], f32, isOutput=False)
    out_d = nc.declare_dram_parameter("out", [1, C], f32, isOutput=True)

    def bcast_ap(dram_h, parts=P):
        # replicate a [N] dram vector across `parts` partitions
        a = dram_h[:]
        return bass.AP(tensor=a.tensor, offset=a.offset,
                       ap=[[0, parts]] + [list(x) for x in a.ap])

    def rep_mid(ap2d, reps):
        # [P, N] -> [P, reps, N] with stride-0 middle axis
        return bass.AP(tensor=ap2d.tensor, offset=ap2d.offset,
                       ap=[list(ap2d.ap[0]), [0, reps], list(ap2d.ap[1])])

    def rep_last(ap2d, reps):
        # [P, N] -> [P, N, reps] with stride-0 last axis
        return bass.AP(tensor=ap2d.tensor, offset=ap2d.offset,
                       ap=[list(ap2d.ap[0]), list(ap2d.ap[1]), [0, reps]])

    from contextlib import ExitStack
    with tile.TileContext(nc) as tc, ExitStack() as ctx:
        wpool = ctx.enter_context(tc.tile_pool(name="wpool", bufs=1))
        bigx = ctx.enter_context(tc.tile_pool(name="bigx", bufs=1))
        qkp = ctx.enter_context(tc.tile_pool(name="qkp", bufs=1))
        vap = ctx.enter_context(tc.tile_pool(name="vap", bufs=1))
        pcp = ctx.enter_context(tc.tile_pool(name="pcp", bufs=3))
        atp = ctx.enter_context(tc.tile_pool(name="atp", bufs=2))
        xnp = ctx.enter_context(tc.tile_pool(name="xnp", bufs=1))
        htp = ctx.enter_context(tc.tile_pool(name="htp", bufs=1))
        xxp = ctx.enter_context(tc.tile_pool(name="xxp", bufs=2))
        tmp = ctx.enter_context(tc.tile_pool(name="tmp", bufs=3))
        small = ctx.enter_context(tc.tile_pool(name="small", bufs=4))
        psA = ctx.enter_context(tc.tile_pool(name="psA", bufs=3, space="PSUM"))
        psS = ctx.enter_context(tc.tile_pool(name="psS", bufs=2, space="PSUM"))
        psV = ctx.enter_context(tc.tile_pool(name="psV", bufs=2, space="PSUM"))

        # ---- persistent loads (host pre-arranged; all contiguous DMAs) ----
        # xA first so layer-0 V can start immediately; ow last (layer-6 only)
        xA = bigx.tile([P, KO, XW], bf16, tag="xA")
        nc.sync.dma_start(xA, d['xT'][:])
        wq_sb = wpool.tile([P, KO, D], bf16, tag="wq")
        wk_sb = wpool.tile([P, KO, D], bf16, tag="wk")
        wv_sb = wpool.tile([P, KO, D], bf16, tag="wv")
        fc1_sb = wpool.tile([P, KO, HD], bf16, tag="fc1")
        fc2_sb = wpool.tile([P, HC, D], bf16, tag="fc2")
        for sb, key in ((wv_sb, 'wvT'), (wq_sb, 'wqT'), (wk_sb, 'wkT')):
            nc.sync.dma_start(sb, d[key][:])
        nc.scalar.dma_start(out=fc1_sb, in_=d['fc1T'][:])
        nc.scalar.dma_start(out=fc2_sb, in_=d['fc2T'][:])
        bq_sb = wpool.tile([P, KO], f32, tag="bq")
        bk_sb = wpool.tile([P, KO], f32, tag="bk")
        nc.sync.dma_start(bq_sb, d['bq'][:])
        nc.sync.dma_start(bk_sb, d['bk'][:])
        bv_bc = wpool.tile([P, D], f32, tag="bv")
        nc.gpsimd.dma_start(out=bv_bc, in_=bcast_ap(d['bv']))
        bv1k_sb = wpool.tile([1, D], f32, tag="bv1k")
        nc.sync.dma_start(bv1k_sb, d['bv1k'][:])
        fc1b_sb = wpool.tile([P, HC], f32, tag="fc1b")
        nc.sync.dma_start(fc1b_sb, d['fc1b'][:])
        fc2b_sb = wpool.tile([1, D], bf16, tag="fc2b")
        nc.sync.dma_start(fc2b_sb, d['fc2b'][:])
        mask_sb = wpool.tile([P, 1280], bf16, tag="mask")
        nc.sync.dma_start(mask_sb, d['mask'][:])
        ow_sb = wpool.tile([P, C, 8, D], bf16, tag="ow")
        nc.gpsimd.dma_start(out=ow_sb, in_=d['owT'][:])
        if affine:
            g_bc = wpool.tile([P, D], f32, tag="g")
            b_bc = wpool.tile([P, D], f32, tag="b")
            nc.gpsimd.dma_start(out=g_bc, in_=bcast_ap(d['lng']))
            nc.gpsimd.dma_start(out=b_bc, in_=bcast_ap(d['lnb']))

        ident = wpool.tile([P, P], bf16, tag="id")
        make_identity(nc, ident)
        ones_col = wpool.tile([P, 1], f32, tag="onc")
        nc.vector.memset(ones_col, 1.0)
        ones1 = wpool.tile([1, P], bf16, tag="on1")
        nc.vector.memset(ones1, 1.0)
        eps_sb = wpool.tile([P, 1], f32, tag="eps")
        nc.vector.memset(eps_sb, EPS)
        # V-totals row [1, 2*(4*65)]; ones-slots hold S (set once)
        vtot_sb = wpool.tile([1, 520], f32, tag="vtot")
        nc.vector.memset(
            vtot_sb.rearrange("p (g i a) -> p g i a", g=2, a=65)[:, :, :, 64:65],
            float(S))
        vtb = wpool.tile([P, 520], f32, tag="vtb")
        red = wpool.tile([P, C, 8], f32, tag="red")
        osb = wpool.tile([1, C], f32, tag="osb")
        scr = wpool.tile([P, D], bf16, tag="scr")
        scr2 = wpool.tile([P, D], bf16, tag="scr2")
        scr3 = wpool.tile([P, D], bf16, tag="scr3")

        # mask views
        m_int = mask_sb[:, 0:256]
        m_e0 = mask_sb[:, 256:512]
        m_e8 = mask_sb[:, 512:768]
        ntri_lo = mask_sb[:, 768:896]
        ntri_lo_e = mask_sb[:, 896:1024]
        ntri_up = mask_sb[:, 1024:1152]
        ntri_up_e = mask_sb[:, 1152:1280]

        # x ping-pong tiles (padded, feature-major)
        xB = bigx.tile([P, KO, XW], bf16, tag="xB")
        nc.gpsimd.memset(xB[:, :, 0:64], 0.0)
        nc.gpsimd.memset(xB[:, :, 64 + S:XW], 0.0)
        x1T = bigx.tile([P, KO, S], bf16, tag="x1T")

        q_pad = [qkp.tile([P, QW], bf16, tag=f"q{mc}", name=f"q{mc}")
                 for mc in range(KO)]
        k_pad = [qkp.tile([P, XW], bf16, tag=f"k{mc}", name=f"k{mc}")
                 for mc in range(KO)]
        for mc in range(KO):
            nc.gpsimd.memset(q_pad[mc][:, 0:128], 0.0)
            nc.gpsimd.memset(q_pad[mc][:, 128 + S:QW], 0.0)
            nc.gpsimd.memset(k_pad[mc][:, 0:64], 0.0)
            nc.gpsimd.memset(k_pad[mc][:, 64 + S:XW], 0.0)
        va = vap.tile([P, 9, 520], bf16, tag="va")
        nc.vector.memset(
            va.rearrange("p s (i a) -> p s i a", a=65)[:, :, :, 64:65], 1.0)

        def layer_norm_to(src_ap, out_tile):
            """LayerNorm src [P,512] -> out_tile [P,512]."""
            st = small.tile([P, 6], f32, tag="st")
            mv = small.tile([P, 2], f32, tag="mv")
            nc.vector.bn_stats(out=st, in_=src_ap)
            nc.vector.bn_aggr(out=mv, in_=st)
            rstd = small.tile([P, 1], f32, tag="rs")
            nc.scalar.activation(out=rstd, in_=mv[:, 1:2], func=AF.Sqrt,
                                 bias=eps_sb[:, 0:1])
            nc.vector.reciprocal(out=rstd, in_=rstd)
            nc.vector.tensor_scalar(out=out_tile, in0=src_ap,
                                    scalar1=mv[:, 0:1], scalar2=rstd,
                                    op0=OP.subtract, op1=OP.mult)
            if affine:
                nc.vector.tensor_tensor(out=out_tile, in0=out_tile, in1=g_bc,
                                        op=OP.mult)
                nc.vector.tensor_tensor(out=out_tile, in0=out_tile, in1=b_bc,
                                        op=OP.add)

        xT = xA
        for L in range(ENC):
            xN = xB if (L % 2 == 0) else xA
            last = (L == ENC - 1)

            # ---------- V projection, 64-shifted token blocks ----------
            # xs reduce for V-totals is split per-ko and interleaved so the
            # DVE never blocks the V bias-adds for long.
            xs32 = small.tile([P, KO], f32, tag="xs")
            for stb in range(9):
                pv = psA.tile([P, 512], f32, tag="pj")
                for ko in range(KO):
                    nc.tensor.matmul(
                        pv, lhsT=xT[:, ko, 128 * stb:128 * stb + 128],
                        rhs=wv_sb[:, ko, :],
                        start=(ko == 0), stop=(ko == KO - 1))
                nc.vector.tensor_tensor(
                    out=va.rearrange("p s (g i a) -> p s g i a",
                                     g=2, a=65)[:, stb, :, :, 0:64],
                    in0=pv.rearrange("p (g i a) -> p g i a", g=2, a=64),
                    in1=bv_bc.rearrange("p (g i a) -> p g i a", g=2, a=64),
                    op=OP.add)
                if stb < KO:
                    nc.vector.reduce_sum(out=xs32[:, stb:stb + 1],
                                         in_=xT[:, stb:stb + 1, :],
                                         axis=mybir.AxisListType.X)

            # ---------- Q/K projections (feature-major, bf16, padded) ----------
            for mc in range(KO if STAGE >= 2 else 0):
                for half in range(2):
                    cs = slice(64 + half * 512, 64 + (half + 1) * 512)
                    pq = psA.tile([P, 512], f32, tag="pj")
                    for ko in range(KO):
                        nc.tensor.matmul(
                            pq, lhsT=wq_sb[:, ko, mc * P:(mc + 1) * P],
                            rhs=xT[:, ko, cs],
                            start=(ko == 0), stop=(ko == KO - 1))
                    nc.scalar.activation(
                        out=q_pad[mc][:, 128 + half * 512:128 + (half + 1) * 512],
                        in_=pq, func=AF.Identity, bias=bq_sb[:, mc:mc + 1])
                    pk = psA.tile([P, 512], f32, tag="pj")
                    for ko in range(KO):
                        nc.tensor.matmul(
                            pk, lhsT=wk_sb[:, ko, mc * P:(mc + 1) * P],
                            rhs=xT[:, ko, cs],
                            start=(ko == 0), stop=(ko == KO - 1))
                    nc.scalar.activation(
                        out=k_pad[mc][:, cs],
                        in_=pk, func=AF.Identity, bias=bk_sb[:, mc:mc + 1])

            # ---------- V totals: (sum_t x) @ wv + S*bv (f32) ----------
            xsr = small.tile([P, KO], bf16, tag="xsr")
            nc.scalar.copy(out=xsr, in_=xs32)
            pvt = psA.tile([P, 512], f32, tag="pj")
            for ko in range(KO):
                nc.tensor.matmul(
                    pvt[0:1, :], lhsT=xsr[:, ko:ko + 1],
                    rhs=wv_sb[:, ko, :],
                    start=(ko == 0), stop=(ko == KO - 1))
            nc.vector.tensor_tensor(
                out=vtot_sb.rearrange("p (g i a) -> p g i a",
                                      g=2, a=65)[:, :, :, 0:64],
                in0=pvt[0:1, :].rearrange("p (g i a) -> p g i a", g=2, a=64),
                in1=bv1k_sb.rearrange("p (g i a) -> p g i a", g=2, a=64),
                op=OP.add)
            nc.gpsimd.partition_broadcast(out_ap=vtb[:, :], in_ap=vtot_sb[0:1, :])

            # ---------- scores + probs (qb-centric key blocks) ----------
            # pc tiles per head-pair hp: [P, 512] = two heads x 256 query cols
            pcs = {}   # (hp, j) -> tile
            a_cur = None

            def attn_part(j, part):
                """Scores/exp/mask for key-block j, heads 4*part..4*part+3."""
                mk = m_int if 0 < j < 8 else (m_e0 if j == 0 else m_e8)
                for h in range(4 * part, 4 * part + 4):
                    hp, sub = h // 2, h % 2
                    hr = slice(64 * sub, 64 * sub + 64)
                    sc = psS.tile([P, 256], f32, tag="s")
                    nc.tensor.matmul(
                        sc, lhsT=k_pad[hp][hr, 128 * j:128 * j + 128],
                        rhs=q_pad[hp][hr, 128 * j:128 * j + 256],
                        start=True, stop=True)
                    pc = pcp.tile([P, 256], bf16, tag=f"pc{h}", name=f"pc{h}")
                    nc.scalar.activation(out=pc, in_=sc, func=AF.Exp, scale=SCALE)
                    nc.vector.tensor_tensor(out=pc, in0=pc, in1=mk, op=OP.mult)
                    pcs[(h, j)] = pc

            def attn_unit(j):
                attn_part(j, 0)
                attn_part(j, 1)

            def attn_av(qb, unit_j=None):
                """AV + normalize for query block qb -> a_tok.
                Interleaves score emission for key-block unit_j so the PE
                always has AV work while Act/DVE produce the next probs."""
                a_tok = atp.tile([P, D], f32, tag="at")
                for g in range(2):  # head group: heads 4g..4g+3
                    if unit_j is not None:
                        attn_part(unit_j, g)
                    pav = psV.tile([P, 260], f32, tag="av")
                    nlo = ntri_lo_e if qb == 0 else ntri_lo
                    nup = ntri_up_e if qb == 7 else ntri_up
                    nc.tensor.matmul(pav, lhsT=nlo,
                                     rhs=va[:, qb, g * 260:(g + 1) * 260],
                                     start=True, stop=False)
                    nc.tensor.matmul(pav, lhsT=nup,
                                     rhs=va[:, qb + 1, g * 260:(g + 1) * 260],
                                     start=False, stop=False)
                    for i in range(4):
                        h = 4 * g + i
                        po = pav[:, i * 65:i * 65 + 65]
                        nc.tensor.matmul(
                            po, lhsT=pcs[(h, qb)][:, 128:256],
                            rhs=va[:, qb, g * 260 + i * 65:g * 260 + i * 65 + 65],
                            start=False, stop=False)
                        nc.tensor.matmul(
                            po, lhsT=pcs[(h, qb + 1)][:, 0:128],
                            rhs=va[:, qb + 1, g * 260 + i * 65:g * 260 + i * 65 + 65],
                            start=False, stop=(i == 3))
                    rc4 = small.tile([P, 4], f32, tag="rc")
                    nc.vector.tensor_scalar_add(
                        out=rc4.rearrange("p (i o) -> p i o", o=1),
                        in0=pav.rearrange("p (i a) -> p i a", a=65)[:, :, 64:65],
                        scalar1=float(S))
                    nc.vector.reciprocal(out=rc4, in_=rc4)
                    asl = a_tok[:, g * 256:(g + 1) * 256]
                    nc.vector.tensor_tensor(
                        out=asl.rearrange("p (i a) -> p i a", a=64),
                        in0=pav.rearrange("p (i a) -> p i a", a=65)[:, :, 0:64],
                        in1=vtb[:, g * 260:(g + 1) * 260].rearrange(
                            "p (i a) -> p i a", a=65)[:, :, 0:64],
                        op=OP.add)
                    nc.vector.tensor_tensor(
                        out=asl.rearrange("p (i a) -> p i a", a=64),
                        in0=asl.rearrange("p (i a) -> p i a", a=64),
                        in1=rep_last(rc4, 64), op=OP.mult)
                return a_tok

            def attn_ln(qb, a_tok):
                """LN1 -> xn (kept for residual) -> x1T feature-major."""
                xn = xnp.tile([P, D], bf16, tag=f"xn{qb}", name=f"xnt{qb}")
                layer_norm_to(a_tok, xn)
                pt = psA.tile([P, 512], bf16, tag="pt", bufs=1)
                for dc in range(KO):
                    nc.tensor.transpose(pt[:, dc * P:(dc + 1) * P],
                                        xn[:, dc * P:(dc + 1) * P], ident)
                nc.scalar.copy(
                    out=x1T[:, :, qb * P:(qb + 1) * P],
                    in_=pt.rearrange("p (ko t) -> p ko t", t=P))
                return xn

            if STAGE <= 2:
                continue
            xns = [None] * 8
            atoks = [None] * 8
            attn_unit(0)
            if STAGE >= 4:
                attn_unit(1)
                if STAGE >= 5:
                    atoks[0] = attn_av(0, unit_j=2)
                    for j in range(3, 9):
                        atoks[j - 2] = attn_av(j - 2, unit_j=j)
                        xns[j - 3] = attn_ln(j - 3, atoks[j - 3])
                    atoks[7] = attn_av(7)
                    xns[6] = attn_ln(6, atoks[6])
                else:
                    for j in range(2, 9):
                        attn_unit(j)
            if STAGE <= 5:
                continue

            # ---------- FFN + residual + LN2 ----------
            hts = [htp.tile([P, 512], bf16, tag=f"h{hc}", name=f"h{hc}")
                   for hc in range(HC)]
            pend = None

            def emit_xpose(tb, xo):
                pt = psA.tile([P, 512], bf16, tag="pt", bufs=1)
                for dc in range(KO):
                    nc.tensor.transpose(pt[:, dc * P:(dc + 1) * P],
                                        xo[:, dc * P:(dc + 1) * P], ident)
                nc.scalar.copy(
                    out=xN[:, :, 64 + tb * P:64 + (tb + 1) * P],
                    in_=pt.rearrange("p (ko t) -> p ko t", t=P))

            def ffn_fc1(half):
                qs = slice(half * 512, (half + 1) * 512)
                for hc in range(HC):
                    ph = psA.tile([P, 512], f32, tag="pj")
                    for ko in range(KO):
                        nc.tensor.matmul(
                            ph, lhsT=fc1_sb[:, ko, hc * P:(hc + 1) * P],
                            rhs=x1T[:, ko, qs],
                            start=(ko == 0), stop=(ko == KO - 1))
                    nc.scalar.activation(out=hts[hc], in_=ph, func=AF.Relu,
                                         bias=fc1b_sb[:, hc:hc + 1])

            for half in range(2):
                ffn_fc1(half)
                if half == 0:
                    # qb7's LN1/transposes land while fc1-half0 runs on PE
                    xns[7] = attn_ln(7, atoks[7])
                for tb2 in range(4):
                    tb = half * 4 + tb2
                    pf = psA.tile([P, 512], f32, tag="pj")
                    # fc2 bias via K=1 matmul (start=True zeroes the bank)
                    nc.tensor.matmul(pf, lhsT=ones1[0:1, :], rhs=fc2b_sb[0:1, :],
                                     start=True, stop=False)
                    for hc in range(HC):
                        nc.tensor.matmul(
                            pf, lhsT=hts[hc][:, tb2 * P:(tb2 + 1) * P],
                            rhs=fc2_sb[:, hc, :],
                            start=False, stop=False)
                    # residual via identity matmul
                    nc.tensor.matmul(pf, lhsT=ident, rhs=xns[tb],
                                     start=False, stop=True)
                    xo = xxp.tile([P, D], bf16, tag="xo")
                    layer_norm_to(pf, xo)
                    if last:
                        # final projection partial: red[p, c, tb]
                        # (work split DVE/Act/GpSimd to avoid a backlog tail)
                        for c in range(C):
                            w = c % 3
                            sc_t = (scr, scr2, scr3)[w]
                            eng = nc.gpsimd if w == 2 else nc.vector
                            eng.tensor_tensor(
                                out=sc_t, in0=xo, in1=ow_sb[:, c, tb, :],
                                op=OP.mult)
                            if w == 0:
                                nc.vector.reduce_sum(
                                    out=red[:, c, tb:tb + 1], in_=sc_t,
                                    axis=mybir.AxisListType.X)
                            else:
                                nc.scalar.activation(
                                    out=sc_t, in_=sc_t, func=AF.Identity,
                                    accum_out=red[:, c, tb:tb + 1])
                    else:
                        # defer transpose one tb so PE never waits on LN2
                        if pend is not None:
                            emit_xpose(*pend)
                        pend = (tb, xo)
            if pend is not None:
                emit_xpose(*pend)
            xT = xN

        # ---------- final cross-partition reduce ----------
        if STAGE <= 8:
            nc.vector.memset(red[:, :, :], 0.0)
        pout = psS.tile([P, 256], f32, tag="s")
        nc.tensor.matmul(pout[0:1, 0:C * 8], lhsT=ones_col[:, 0:1],
                         rhs=red.rearrange("p c t -> p (c t)"),
                         start=True, stop=True)
        nc.vector.reduce_sum(
            out=osb, in_=pout[0:1, 0:C * 8].rearrange("p (c t) -> p c t", t=8),
            axis=mybir.AxisListType.X)
        nc.sync.dma_start(out_d[:], osb)

    nc.compile()
    return nc


def _prep(inputs):
    """Host-side input prep shared across cores. Returns (common, per_core, affine)."""
    import ml_dtypes
    bf = ml_dtypes.bfloat16

    emb = np.asarray(inputs['emb'], dtype=np.float32)
    idx = np.asarray(inputs['inputs'])
    pos = np.arange(S, dtype=np.float32)[:, None]
    div = np.exp(-np.log(10000.0) * np.arange(0, D, 2, dtype=np.float32) / D)
    ang = pos * div
    pe = np.zeros((S, D), dtype=np.float32)
    pe[:, 0::2] = np.sin(ang)
    pe[:, 1::2] = np.cos(ang)
    x0 = emb[idx] + pe[None]  # [B, S, D]

    # masks (bf16): interior M[p,c] = (p < c) & (p >= c-128) for key block
    # B_j vs query cols [128j-128, 128j+128)
    p_ = np.arange(P)[:, None]
    c_ = np.arange(256)[None, :]
    m_int = ((p_ < c_) & (p_ >= c_ - 128)).astype(np.float32)
    m_e0 = m_int * (p_ >= 64)     # j=0: keys [-64, 64), first 64 partitions fake
    m_e8 = m_int * (p_ < 64)      # j=8: keys [960, 1088), last 64 fake
    c128 = np.arange(128)[None, :]
    tri_lo = (p_ >= c128).astype(np.float32)
    tri_up = (p_ < c128).astype(np.float32)
    ntri_lo = -tri_lo
    ntri_lo_e = -(tri_lo * (p_ >= 64))
    ntri_up = -tri_up
    ntri_up_e = -(tri_up * (p_ < 64))
    mask = np.concatenate(
        [m_int, m_e0, m_e8, ntri_lo, ntri_lo_e, ntri_up, ntri_up_e],
        axis=1).astype(bf)

    ln_g = np.asarray(inputs['ln_g'], dtype=np.float32)
    ln_b = np.asarray(inputs['ln_b'], dtype=np.float32)
    affine = not (np.all(ln_g == 1.0) and np.all(ln_b == 0.0))

    def wmaj(wT, ko):  # [D, N] feature-major -> [P, ko, N]
        N = wT.shape[1]
        return np.ascontiguousarray(
            wT.reshape(ko, P, N).transpose(1, 0, 2)).astype(bf)

    out_w = np.asarray(inputs['out_w'], dtype=np.float32)
    owT = np.ascontiguousarray(
        out_w.reshape(C, 8, P, D).transpose(2, 0, 1, 3)).astype(bf)  # [P,C,8,D]

    bq = np.asarray(inputs['bq'], np.float32)
    bk = np.asarray(inputs['bk'], np.float32)
    bv = np.asarray(inputs['bv'], np.float32)
    fc1b = np.asarray(inputs['fc1_b'], np.float32)

    common = {
        'wqT': wmaj(np.asarray(inputs['wq'], np.float32).T, KO),
        'wkT': wmaj(np.asarray(inputs['wk'], np.float32).T, KO),
        'wvT': wmaj(np.asarray(inputs['wv'], np.float32).T, KO),
        'fc1T': wmaj(np.asarray(inputs['fc1_w'], np.float32).T, KO),
        'fc2T': wmaj(np.asarray(inputs['fc2_w'], np.float32).T, HC),
        'bq': np.ascontiguousarray(bq.reshape(KO, P).T),
        'bk': np.ascontiguousarray(bk.reshape(KO, P).T),
        'bv': np.ascontiguousarray(bv),
        'bv1k': np.ascontiguousarray(bv[None, :] * float(S)),
        'fc1b': np.ascontiguousarray(fc1b.reshape(HC, P).T),
        'fc2b': np.ascontiguousarray(
            np.asarray(inputs['fc2_b'], np.float32)[None, :]).astype(bf),
        'mask': mask,
        'owT': owT,
    }
    if affine:
        common['lng'] = np.ascontiguousarray(ln_g)
        common['lnb'] = np.ascontiguousarray(ln_b)
    per_core = []
    for b in range(B):
        xp = np.zeros((D, XW), dtype=np.float32)
        xp[:, 64:64 + S] = x0[b].T
        per_core.append({'xT': np.ascontiguousarray(
            xp.reshape(KO, P, XW).transpose(1, 0, 2)).astype(bf)})
    return common, per_core, affine


def kernel(**inputs):
    global LAST_EXEC_NS, LAST_RESULTS
    from concourse.bass_utils import run_bass_kernel_spmd

    common, per_core, affine = _prep(inputs)
    if affine not in _CACHE:
        _CACHE[affine] = _build(affine)
    nc = _CACHE[affine]

    in_maps = [dict(common, **pc) for pc in per_core]
    res = run_bass_kernel_spmd(nc, in_maps, list(range(B)), trace=TRACE)
    LAST_EXEC_NS = res.exec_time_ns
    LAST_RESULTS = res
    out = np.stack([res.results[b]["out"][0] for b in range(B)], axis=0)
    out = out + np.asarray(inputs['out_b'], np.float32)[None, :]
    return out.astype(np.float32)
